# revision 5
# baseline (speedup 1.0000x reference)
"""FDoG fully on-device for Trainium2 (8 cores, column-parallel).

Everything in [partition=column, free=row] layout. Per core: a 384-col slab
(halo included) arrives as input; sobel/tang/smag (Newton-refined sqrt and
divisions), the 6-step ETF relaxation, the 19-tap DoG (GPSIMD ap_gather over
per-16-col-group image windows), the 2x30-step streamline integration (fused
(etfx,etfy,dog) d=3 gathers, replicated state), and the final threshold all
run on device.  Output: u8 mask, decoded and assembled on host.
"""

import math
import os
import time

import numpy as np

# ---------------------------------------------------------------- constants
MU = 10
ITERATIONS = 3
SIGMA_C = 3.0
SIGMA_S = SIGMA_C * 1.6
SIGMA_M = 10.0
RHO = 0.99
DELTA = 1.0
MAX_T = 9
MAX_S = 30

B, X, Y = 2, 1024, 1024
N_CORES = 8
CPI = 4

F = 1064                # ETF tile free dim (rows + zero tail)
RV = 1034               # ETF compute rows
NT = 3                  # slab partition tiles
SLAB = 384              # slab cols;  s0 = c0 - 40
IMROWS = 1042           # img input rows: global rows -9..1032

FPL_C, FPL_R = 384, 1120          # fused plane: col c0-30+P, row R-30
DW_E = 34 * 530                   # DoG window elems (34 cols x 530 rows)
SW_E = 76 * 124                   # streamline window elems (76 x 124)
CBIG = 2.0 ** 23


def _gauss(v, sigma):
    return math.exp(-v ** 2 / (2.0 * sigma ** 2)) / (math.sqrt(2.0 * math.pi) * sigma)


W0_S = np.float32(_gauss(0, SIGMA_M))
WS_S = [np.float32(_gauss(s, SIGMA_M)) for s in range(1, MAX_S + 1)]
WT_D = [np.float32(_gauss(t, SIGMA_C) - RHO * _gauss(t, SIGMA_S))
        for t in range(-MAX_T, MAX_T + 1)]
TOTD = np.float32(sum(_gauss(t, SIGMA_C) - RHO * _gauss(t, SIGMA_S)
                      for t in range(-MAX_T, MAX_T + 1)))
INV_TOTD = np.float32(1.0 / np.float64(TOTD))
CUT_ACC = np.uint32(0xbe9e1cea).view(np.float32)  # acc>=CUT <=> keep (bisected)

_CACHE = {}
DBG = bool(os.environ.get("BASSK_DBG"))


# ================================================================ bass build
def _build():
    import concourse.bacc as bacc
    import concourse.mybir as mybir
    import concourse.tile as tile
    from concourse.bass_types import AP

    f32 = mybir.dt.float32
    i16 = mybir.dt.int16
    i32 = mybir.dt.int32
    u8 = mybir.dt.uint8
    Alu = mybir.AluOpType
    Act = mybir.ActivationFunctionType
    Ax = mybir.AxisListType

    nc = bacc.Bacc("TRN2", target_bir_lowering=False, debug=False,
                   enable_asserts=False, num_devices=N_CORES)

    img = nc.dram_tensor("img", [SLAB, IMROWS], f32, kind="ExternalInput").ap()
    cc = nc.dram_tensor("cconst", [128, 20], f32, kind="ExternalInput").ap()
    spat = nc.dram_tensor("scatpat", [128, 1024], i16, kind="ExternalInput").ap()
    mask_o = nc.dram_tensor("mask", [256, 1024], u8, kind="ExternalOutput").ap()
    if DBG:
        dtng_o = nc.dram_tensor("dtng", [3 * SLAB, F], f32, kind="ExternalOutput").ap()
        dfpl_o = nc.dram_tensor("dfpl", [FPL_C, FPL_R, 3], f32, kind="ExternalOutput").ap()

    tng = nc.dram_tensor("tng", [3 * SLAB, F], f32, kind="Internal").ap()
    fpl_t = nc.dram_tensor("fpl", [FPL_C, FPL_R, 3], f32, kind="Internal")
    fpl = fpl_t.ap()
    grin = nc.dram_tensor("grin", [1, 1], f32, kind="Internal").ap()
    grout = nc.dram_tensor("grout", [1, 1], f32, kind="Internal").ap()
    gtmp = nc.dram_tensor("gtmp", [3, 128], f32, kind="Internal").ap()
    gbd = nc.dram_tensor("gbd", [2, 128], f32, kind="Internal").ap()

    with tile.TileContext(nc) as tc:
        # ---------------- stage 1: sobel, mag, tang0, smag ----------------
        with tc.tile_pool(name="s1", bufs=1) as pool:
            imt = [pool.tile([128, F], f32, name=f"imt{t}", tag=f"imt{t}") for t in range(NT)]
            imL = [pool.tile([128, F], f32, name=f"imL{t}", tag=f"imL{t}") for t in range(NT)]
            imR = [pool.tile([128, F], f32, name=f"imR{t}", tag=f"imR{t}") for t in range(NT)]
            gx = [pool.tile([128, F], f32, name=f"gx{t}", tag=f"gx{t}") for t in range(NT)]
            gy = [pool.tile([128, F], f32, name=f"gy{t}", tag=f"gy{t}") for t in range(NT)]
            ta = [pool.tile([128, F], f32, name=f"ta{t}", tag=f"ta{t}") for t in range(NT)]
            tb = [pool.tile([128, F], f32, name=f"tb{t}", tag=f"tb{t}") for t in range(NT)]
            tcl = [pool.tile([128, F], f32, name=f"tc{t}", tag=f"tc{t}") for t in range(NT)]
            sc = [pool.tile([128, F], f32, name=f"sc{k}", tag=f"sc{k}") for k in range(6)]
            mx = [pool.tile([128, 1], f32, name=f"mx{t}", tag=f"mx{t}") for t in range(NT)]
            gall = pool.tile([1, 384], f32, name="gall", tag="gall")
            g1 = pool.tile([1, 1], f32, name="g1", tag="g1")
            gsc = pool.tile([1, 4], f32, name="gsc", tag="gsc")
            gb = pool.tile([1, 256], f32, name="gb", tag="gb")
            bc = pool.tile([128, 4], f32, name="bc", tag="bc")

            for t in range(NT):
                nc.vector.memset(imt[t][:], 0.0)
                nc.vector.memset(imL[t][:], 0.0)
                nc.vector.memset(imR[t][:], 0.0)
                nc.vector.memset(gx[t][:], 0.0)
                nc.vector.memset(gy[t][:], 0.0)
                # rows -1..1024 at free 0..1025  (img free = row+9)
                nc.sync.dma_start(imt[t][:, 0:1026],
                                  img[128 * t:128 * (t + 1), 8:1034])
            # col-shifted copies: imL[p]=img col-1, imR[p]=img col+1
            for t in range(NT):
                nc.sync.dma_start(imL[t][1:128, 0:1026], imt[t][0:127, 0:1026])
                if t > 0:
                    nc.sync.dma_start(imL[t][0:1, 0:1026], imt[t - 1][127:128, 0:1026])
                nc.sync.dma_start(imR[t][0:127, 0:1026], imt[t][1:128, 0:1026])
                if t + 1 < NT:
                    nc.sync.dma_start(imR[t][127:128, 0:1026], imt[t + 1][0:1, 0:1026])

            v0 = slice(0, 1024)
            for t in range(NT):
                i0 = lambda x: x[t][:, 0:1024]
                i1 = lambda x: x[t][:, 1:1025]
                i2 = lambda x: x[t][:, 2:1026]
                A, Bt, C2 = ta[t], tb[t], tcl[t]
                # gx, numpy order
                nc.vector.scalar_tensor_tensor(A[:, v0], i0(imt), 2.0, i0(imL),
                                               op0=Alu.mult, op1=Alu.add)
                nc.vector.tensor_add(Bt[:, v0], A[:, v0], i0(imR))
                nc.vector.tensor_sub(A[:, v0], i2(imL), Bt[:, v0])
                nc.vector.scalar_tensor_tensor(Bt[:, v0], i2(imt), 2.0, A[:, v0],
                                               op0=Alu.mult, op1=Alu.add)
                nc.vector.tensor_add(gx[t][:, v0], Bt[:, v0], i2(imR))
                # gy
                nc.vector.scalar_tensor_tensor(A[:, v0], i1(imL), 2.0, i0(imL),
                                               op0=Alu.mult, op1=Alu.add)
                nc.vector.tensor_add(Bt[:, v0], A[:, v0], i2(imL))
                nc.vector.tensor_sub(A[:, v0], i0(imR), Bt[:, v0])
                nc.vector.scalar_tensor_tensor(Bt[:, v0], i1(imR), 2.0, A[:, v0],
                                               op0=Alu.mult, op1=Alu.add)
                nc.vector.tensor_add(gy[t][:, v0], Bt[:, v0], i2(imR))
                # mag = sqrt(gx^2+gy^2), Dekker-exact residual correction
                MUL = nc.vector.tensor_mul
                SUB = nc.vector.tensor_sub
                ADD = nc.vector.tensor_add
                TS = nc.vector.tensor_scalar
                STT = nc.vector.scalar_tensor_tensor
                s0v, s1v = sc[0][:, v0], sc[1][:, v0]
                Av, Bv, Cv = A[:, v0], Bt[:, v0], C2[:, v0]
                it_, il_, ir_ = imt[t][:, v0], imL[t][:, v0], imR[t][:, v0]
                MUL(it_, gx[t][:, v0], gx[t][:, v0])
                MUL(il_, gy[t][:, v0], gy[t][:, v0])
                ADD(Cv, it_, il_)                                   # C = m2
                TS(it_, Cv, 0.0, None, op0=Alu.is_equal)
                ADD(Bv, Cv, it_)                                    # B = m2z
                nc.scalar.activation(Av, Bv, Act.Sqrt)
                nc.vector.reciprocal(Av, Av)                        # r0
                MUL(it_, Bv, Av)
                STT(it_, it_, 0.5, Av, op0=Alu.mult, op1=Alu.mult)
                TS(it_, it_, -1.0, 1.5, op0=Alu.mult, op1=Alu.add)
                MUL(Av, Av, it_)                                    # A = r1
                MUL(it_, Cv, Av)                                    # imt = mag0
                # Dekker square of mag0
                TS(il_, it_, 4097.0, None, op0=Alu.mult)
                SUB(ir_, il_, it_)
                SUB(il_, il_, ir_)                                  # imL = hi
                SUB(ir_, it_, il_)                                  # imR = lo
                MUL(Bv, it_, it_)                                   # B = phi
                MUL(s0v, il_, il_)
                SUB(s0v, s0v, Bv)
                MUL(s1v, il_, ir_)
                TS(s1v, s1v, 2.0, None, op0=Alu.mult)
                ADD(s0v, s0v, s1v)
                MUL(s1v, ir_, ir_)
                ADD(s0v, s0v, s1v)                                  # s0 = err
                SUB(s1v, Cv, Bv)
                SUB(s1v, s1v, s0v)                                  # s1 = e exact
                STT(s1v, s1v, 0.5, Av, op0=Alu.mult, op1=Alu.mult)
                ADD(Cv, it_, s1v)                                   # C = mag
                nc.vector.memset(C2[:, 1024:F], 0.0)
                # col-max over rows
                nc.vector.tensor_reduce(mx[t][:, 0:1], C2[:, 0:1024],
                                        axis=Ax.X, op=Alu.max)
                nc.sync.dma_start(gtmp[t:t + 1, :], mx[t][:, 0:1])

            nc.sync.dma_start(gall[:], gtmp.rearrange("a b -> (a b)").unsqueeze(0))
            nc.vector.tensor_reduce(g1[:, 0:1], gall[:, 40:296], axis=Ax.X, op=Alu.max)
            nc.sync.dma_start(grin, g1[:, 0:1])
            if os.environ.get("BASSK_NOCC"):
                nc.sync.dma_start(grout, grin)
            else:
                nc.gpsimd.collective_compute(
                    "AllReduce", Alu.max,
                    replica_groups=[[0, 1, 2, 3, 4, 5, 6, 7]],
                    ins=[grin], outs=[grout])
            nc.sync.dma_start(gsc[:, 0:1], grout)
            # refined reciprocal of gmax on [1,1]
            nc.vector.reciprocal(gsc[:, 1:2], gsc[:, 0:1])
            nc.vector.tensor_mul(gsc[:, 2:3], gsc[:, 0:1], gsc[:, 1:2])
            nc.vector.tensor_scalar(gsc[:, 2:3], gsc[:, 2:3], -1.0, 2.0,
                                    op0=Alu.mult, op1=Alu.add)
            nc.vector.tensor_mul(gsc[:, 1:2], gsc[:, 1:2], gsc[:, 2:3])
            # broadcast gmax, rinv to 128 partitions via log-doubling + DRAM
            nc.vector.tensor_copy(gb[:, 0:1], gsc[:, 0:1])
            nc.vector.tensor_copy(gb[:, 128:129], gsc[:, 1:2])
            k = 1
            while k < 128:
                nc.vector.tensor_copy(gb[:, k:2 * k], gb[:, 0:k])
                nc.vector.tensor_copy(gb[:, 128 + k:128 + 2 * k], gb[:, 128:128 + k])
                k *= 2
            nc.sync.dma_start(gbd.rearrange("j p -> (j p)").unsqueeze(0), gb[:])
            nc.sync.dma_start(bc[:, 0:2], gbd.rearrange("j p -> p j"))
            gmax_ap = bc[:, 0:1]
            rinv_ap = bc[:, 1:2]
            # Veltkamp split of gmax -> bc[:,2]=ghi, bc[:,3]=glo
            nc.vector.tensor_scalar(bc[:, 2:3], bc[:, 0:1], 4097.0, None, op0=Alu.mult)
            nc.vector.tensor_sub(bc[:, 3:4], bc[:, 2:3], bc[:, 0:1])
            nc.vector.tensor_sub(bc[:, 2:3], bc[:, 2:3], bc[:, 3:4])
            nc.vector.tensor_sub(bc[:, 3:4], bc[:, 0:1], bc[:, 2:3])
            ghi_ap = bc[:, 2:3]
            glo_ap = bc[:, 3:4]

            MUL = nc.vector.tensor_mul
            SUB = nc.vector.tensor_sub
            ADD = nc.vector.tensor_add
            TS = nc.vector.tensor_scalar

            def dekker_div(q0, x, th, tl, tden, r, phi, qh, ql, er, e):
                """q0 <- correctly-rounded-ish x / tden (q0 = fl(x*r) on entry).
                th/tl: split of tden; phi,qh,ql,er,e: scratch views."""
                MUL(phi, q0, tden)
                TS(qh, q0, 4097.0, None, op0=Alu.mult)
                SUB(ql, qh, q0)
                SUB(qh, qh, ql)
                SUB(ql, q0, qh)
                MUL(er, qh, th)
                SUB(er, er, phi)
                MUL(e, qh, tl)
                ADD(er, er, e)
                MUL(e, ql, th)
                ADD(er, er, e)
                MUL(e, ql, tl)
                ADD(er, er, e)
                SUB(e, x, phi)
                SUB(e, e, er)
                MUL(e, e, r)
                ADD(q0, q0, e)

            for t in range(NT):
                A, Bt, C2 = ta[t], tb[t], tcl[t]
                Av, Bv, Cv = A[:, v0], Bt[:, v0], C2[:, v0]
                it_, il_, ir_ = imt[t][:, v0], imL[t][:, v0], imR[t][:, v0]
                s0v, s1v, s2v = sc[0][:, v0], sc[1][:, v0], sc[2][:, v0]
                s3v, s4v, s5v = sc[3][:, v0], sc[4][:, v0], sc[5][:, v0]
                # tmag = mag + (mag==0) -> B; CR reciprocal -> A
                TS(Av, Cv, 0.0, None, op0=Alu.is_equal)
                ADD(Bv, Cv, Av)
                nc.vector.reciprocal(Av, Bv)
                # split tmag -> imt=th, imL=tl
                TS(it_, Bv, 4097.0, None, op0=Alu.mult)
                SUB(il_, it_, Bv)
                SUB(it_, it_, il_)
                SUB(il_, Bv, it_)
                # t0y = gx/tmag -> imR
                MUL(ir_, gx[t][:, v0], Av)
                dekker_div(ir_, gx[t][:, v0], it_, il_, Bv, Av,
                           s0v, s1v, s2v, s3v, s4v)
                nc.vector.memset(imR[t][:, 1024:F], 0.0)
                nc.sync.dma_start(tng[SLAB + 128 * t:SLAB + 128 * (t + 1), :], imR[t][:])
                # t0x = (-gy)/tmag -> gy
                TS(s5v, gy[t][:, v0], -1.0, None, op0=Alu.mult)
                MUL(gy[t][:, v0], s5v, Av)
                dekker_div(gy[t][:, v0], s5v, it_, il_, Bv, Av,
                           s0v, s1v, s2v, s3v, s4v)
                nc.vector.memset(gy[t][:, 1024:F], 0.0)
                nc.sync.dma_start(tng[128 * t:128 * (t + 1), :], gy[t][:])
                # smag = mag/gmax -> A (phi->B via scalar mult)
                TS(Av, Cv, rinv_ap, None, op0=Alu.mult)          # q0
                TS(Bv, Av, gmax_ap, None, op0=Alu.mult)          # phi
                TS(s1v, Av, 4097.0, None, op0=Alu.mult)
                SUB(s2v, s1v, Av)
                SUB(s1v, s1v, s2v)                                # qh
                SUB(s2v, Av, s1v)                                 # ql
                TS(s3v, s1v, ghi_ap, None, op0=Alu.mult)
                SUB(s3v, s3v, Bv)
                TS(s4v, s1v, glo_ap, None, op0=Alu.mult)
                ADD(s3v, s3v, s4v)
                TS(s4v, s2v, ghi_ap, None, op0=Alu.mult)
                ADD(s3v, s3v, s4v)
                TS(s4v, s2v, glo_ap, None, op0=Alu.mult)
                ADD(s3v, s3v, s4v)                                # err
                SUB(s4v, Cv, Bv)
                SUB(s4v, s4v, s3v)                                # e exact
                TS(s4v, s4v, rinv_ap, None, op0=Alu.mult)
                ADD(Cv, Av, s4v)                                  # smag -> C
                nc.vector.memset(C2[:, 1024:F], 0.0)
                nc.sync.dma_start(tng[2 * SLAB + 128 * t:2 * SLAB + 128 * (t + 1), :],
                                  C2[:])

        if DBG:
            with tc.tile_pool(name="dbg1", bufs=1) as pool:
                dt_ = pool.tile([128, F], f32, name="dbg_t", tag="dbg_t")
                for k2 in range(9):
                    nc.sync.dma_start(dt_[:], tng[128 * k2:128 * (k2 + 1), :])
                    nc.sync.dma_start(dtng_o[128 * k2:128 * (k2 + 1), :], dt_[:])

        # ---------------- stage 2: ETF relaxation (baseline code) ---------
        with tc.tile_pool(name="etf", bufs=1) as pool:
            tgx = [pool.tile([128, F], f32, name=f"tgx{t}", tag=f"tgx{t}") for t in range(NT)]
            tgy = [pool.tile([128, F], f32, name=f"tgy{t}", tag=f"tgy{t}") for t in range(NT)]
            smg = [pool.tile([128, F], f32, name=f"smg{t}", tag=f"smg{t}") for t in range(NT)]
            smgs = [pool.tile([128, F], f32, name=f"smgs{t}", tag=f"smgs{t}") for t in range(NT)]
            nx = [pool.tile([128, F], f32, name=f"nx{t}", tag=f"nx{t}") for t in range(NT)]
            ny = [pool.tile([128, F], f32, name=f"ny{t}", tag=f"ny{t}") for t in range(NT)]
            sf = [pool.tile([128, F], f32, name=f"sf{t}", tag=f"sf{t}") for t in range(NT)]
            m2 = [pool.tile([128, F], f32, name=f"m2{t}", tag=f"m2{t}") for t in range(NT)]
            dts = [pool.tile([128, 4], f32, name=f"dt{t}", tag=f"dt{t}") for t in range(NT)]
            pp = [pool.tile([128, 8], f32, name=f"pp{t}", tag=f"pp{t}") for t in range(NT)]
            hx = [pool.tile([128, F], f32, name=f"hx{t}", tag=f"hx{t}") for t in range(NT)]
            hy = [pool.tile([128, F], f32, name=f"hy{t}", tag=f"hy{t}") for t in range(NT)]

            for t in range(NT):
                nc.vector.memset(nx[t][:], 0.0)
                nc.vector.memset(ny[t][:], 0.0)
                nc.vector.memset(smgs[t][:], 0.0)
                nc.vector.memset(hx[t][:], 0.0)
                nc.vector.memset(hy[t][:], 0.0)
                nc.sync.dma_start(tgx[t][:], tng[128 * t:128 * (t + 1), :])
                nc.sync.dma_start(tgy[t][:], tng[SLAB + 128 * t:SLAB + 128 * (t + 1), :])
                nc.sync.dma_start(smg[t][:], tng[2 * SLAB + 128 * t:2 * SLAB + 128 * (t + 1), :])

            def hshift(dst, src):
                for t in range(NT):
                    nc.sync.dma_start(dst[t][0:118, :], src[t][10:128, :])
                    if t + 1 < NT:
                        nc.sync.dma_start(dst[t][118:128, :], src[t + 1][0:10, :])

            hshift(smgs, smg)

            for _ in range(ITERATIONS):
                for ori in ("V", "H"):
                    if ori == "H":
                        hshift(hx, tgx)
                        hshift(hy, tgy)
                    for t in range(NT):
                        if ori == "V":
                            tYx = tgx[t][:, 10:10 + RV]
                            tYy = tgy[t][:, 10:10 + RV]
                            sY = smg[t][:, 10:10 + RV]
                        else:
                            tYx = hx[t][:, 0:RV]
                            tYy = hy[t][:, 0:RV]
                            sY = smgs[t][:, 0:RV]
                        v = slice(0, RV)
                        nc.vector.tensor_mul(m2[t][:, v], tgx[t][:, v], tYx)
                        nc.vector.tensor_reduce(
                            pp[t][:, 0:8],
                            m2[t][:, 0:1024].rearrange("p (a b) -> p a b", b=128),
                            axis=Ax.X, op=Alu.add)
                        nc.vector.tensor_reduce(
                            dts[t][:, 0:1], pp[t][:, 0:8], axis=Ax.X, op=Alu.add)
                        nc.vector.tensor_mul(m2[t][:, v], tgy[t][:, v], tYy)
                        nc.vector.tensor_reduce(
                            pp[t][:, 0:8],
                            m2[t][:, 0:1024].rearrange("p (a b) -> p a b", b=128),
                            axis=Ax.X, op=Alu.add)
                        nc.vector.tensor_reduce(
                            dts[t][:, 1:2], pp[t][:, 0:8], axis=Ax.X, op=Alu.add)
                        nc.vector.tensor_scalar_mul(dts[t][:, 0:2], dts[t][:, 0:2], 0.5)
                        nc.vector.tensor_sub(sf[t][:, v], sY, smg[t][:, v])
                        nc.vector.tensor_scalar_add(sf[t][:, v], sf[t][:, v], 1.0)
                        nc.vector.tensor_mul(nx[t][:, v], tYx, sf[t][:, v])
                        nc.vector.tensor_scalar_mul(nx[t][:, v], nx[t][:, v],
                                                    dts[t][:, 0:1])
                        nc.vector.tensor_mul(ny[t][:, v], tYy, sf[t][:, v])
                        nc.vector.tensor_scalar_mul(ny[t][:, v], ny[t][:, v],
                                                    dts[t][:, 1:2])
                        nc.vector.tensor_mul(m2[t][:, v], nx[t][:, v], nx[t][:, v])
                        nc.vector.tensor_mul(sf[t][:, v], ny[t][:, v], ny[t][:, v])
                        nc.vector.tensor_add(m2[t][:, v], m2[t][:, v], sf[t][:, v])
                    for t in range(NT):
                        v = slice(0, RV)
                        nc.scalar.activation(sf[t][:, v], m2[t][:, v], Act.Sqrt)
                        nc.vector.tensor_scalar(hx[t][:, v], sf[t][:, v], 0.0,
                                                None, op0=Alu.is_equal)
                        nc.vector.tensor_add(sf[t][:, v], sf[t][:, v], hx[t][:, v])
                        nc.vector.reciprocal(sf[t][:, v], sf[t][:, v])
                        nc.vector.tensor_mul(hx[t][:, v], m2[t][:, v], sf[t][:, v])
                        nc.vector.scalar_tensor_tensor(
                            hx[t][:, v], hx[t][:, v], 0.5, sf[t][:, v],
                            op0=Alu.mult, op1=Alu.mult)
                        nc.vector.tensor_scalar(hx[t][:, v], hx[t][:, v], -1.0,
                                                1.5, op0=Alu.mult, op1=Alu.add)
                        nc.vector.tensor_mul(m2[t][:, v], hx[t][:, v], sf[t][:, v])
                        nc.vector.tensor_mul(tgx[t][:, v], nx[t][:, v], m2[t][:, v])
                        nc.vector.tensor_mul(tgy[t][:, v], ny[t][:, v], m2[t][:, v])

            # write etf into fused plane: fpl[P, r+30, 0/1], P = slab_col - 10
            for t, (sp0, P0, npart) in enumerate(
                    [(10, 0, 118), (0, 118, 128), (0, 246, 128)]):
                for k2, tt in ((0, tgx), (1, tgy)):
                    dst = AP(tensor=fpl_t, offset=P0 * FPL_R * 3 + 30 * 3 + k2,
                             ap=[[FPL_R * 3, npart], [3, 1024]])
                    nc.sync.dma_start(dst, tt[t][sp0:sp0 + npart, 0:1024])

        # ---------------- stage 3: DoG -----------------------------------
        with tc.tile_pool(name="dog", bufs=1) as pool:
            winD = pool.tile([128, DW_E], f32, name="winD", tag="winD")
            accD = pool.tile([128, 8192], f32, name="accD", tag="accD")
            gout = pool.tile([128, 8192], f32, name="goutD", tag="goutD")
            exs = pool.tile([128, 1024], f32, name="exs", tag="exs")
            eys = pool.tile([128, 1024], f32, name="eys", tag="eys")
            rgn0 = pool.tile([128, 1024], i32, name="rgn0", tag="rgn0")
            rgn0f = pool.tile([128, 1024], f32, name="rgn0f", tag="rgn0f")
            cgn = pool.tile([128, 512], f32, name="cgn", tag="cgn")
            tr = pool.tile([128, 512], f32, name="trD", tag="trD")
            tcc = pool.tile([128, 512], f32, name="tcD", tag="tcD")
            lif = pool.tile([128, 512], f32, name="lifD", tag="lifD")
            idxD = pool.tile([128, 512], i16, name="idxD", tag="idxD")
            ccD = pool.tile([128, 20], f32, name="ccD", tag="ccD")

            nc.sync.dma_start(ccD[:], cc)
            nc.gpsimd.iota(rgn0[:], pattern=[[1, 1024]], base=0, channel_multiplier=0)
            nc.vector.tensor_copy(rgn0f[:], rgn0[:])

            for st in range(3):
                clo_ap = ccD[:, st:st + 1]
                chi_ap = ccD[:, 3 + st:4 + st]
                off_ap = ccD[:, 6 + st:7 + st]
                cg_ap = ccD[:, 9 + st:10 + st]
                # etf strips for per-pixel tangent
                nc.sync.dma_start(
                    exs[:], fpl[128 * st:128 * (st + 1), 30:1054, 0:1].squeeze(2))
                nc.sync.dma_start(
                    eys[:], fpl[128 * st:128 * (st + 1), 30:1054, 1:2].squeeze(2))
                nc.vector.tensor_scalar(cgn[:], rgn0f[:, 0:512], 0.0, cg_ap,
                                        op0=Alu.mult, op1=Alu.add)
                for sl in range(2):
                    rb = 512 * sl
                    # stage window: 16 lane-DMAs; group g: img[cbe(g), +34) x [rb, rb+530)
                    wv = winD[:].rearrange("(g q) e -> q g e", q=16)
                    uniform = (128 * st + 16 * 7 + 1) <= 350
                    for q in range(16):
                        if uniform:
                            sap = AP(tensor=img.tensor,
                                     offset=(128 * st + 1) * IMROWS + rb,
                                     ap=[[16 * IMROWS, 8], [IMROWS, 34], [1, 530]])
                            nc.sync.dma_start(wv[q], sap)
                        else:
                            # per-group clamped bases (st=2 edge groups)
                            for g in range(8):
                                be = min(128 * st + 16 * g + 1, 350)
                                s1 = AP(tensor=img.tensor, offset=be * IMROWS + rb,
                                        ap=[[IMROWS, 34], [1, 530]])
                                nc.sync.dma_start(wv[q][g:g + 1, :], s1.unsqueeze(0))
                    ex_v = exs[:, rb:rb + 512]
                    ey_v = eys[:, rb:rb + 512]
                    rg_v = rgn0f[:, rb:rb + 512]
                    nc.vector.memset(accD[:], 0.0)
                    for ti, t in enumerate(range(-MAX_T, MAX_T + 1)):
                        w = WT_D[ti]
                        # p0 = r + (-ey)*t ; p1 = c + ex*t
                        nc.vector.scalar_tensor_tensor(tr[:], ey_v, -float(t), rg_v,
                                                       op0=Alu.mult, op1=Alu.add)
                        nc.vector.scalar_tensor_tensor(tcc[:], ex_v, float(t), cgn[:],
                                                       op0=Alu.mult, op1=Alu.add)
                        nc.vector.tensor_scalar(tr[:], tr[:], 0.0, 1023.0,
                                                op0=Alu.max, op1=Alu.min)
                        nc.vector.tensor_scalar(tcc[:], tcc[:], 0.0, 1023.0,
                                                op0=Alu.max, op1=Alu.min)
                        nc.vector.tensor_scalar(tr[:], tr[:], CBIG, CBIG,
                                                op0=Alu.add, op1=Alu.subtract)
                        nc.vector.tensor_scalar(tcc[:], tcc[:], CBIG, CBIG,
                                                op0=Alu.add, op1=Alu.subtract)
                        nc.vector.tensor_scalar(tr[:], tr[:], float(rb - 9),
                                                float(rb + 520), op0=Alu.max, op1=Alu.min)
                        nc.vector.tensor_scalar(tcc[:], tcc[:], clo_ap, chi_ap,
                                                op0=Alu.max, op1=Alu.min)
                        nc.vector.scalar_tensor_tensor(lif[:], tcc[:], 530.0, tr[:],
                                                       op0=Alu.mult, op1=Alu.add)
                        nc.vector.tensor_scalar(lif[:], lif[:], off_ap, float(rb - 9),
                                                op0=Alu.subtract, op1=Alu.subtract)
                        nc.vector.tensor_copy(idxD[:], lif[:])
                        nc.gpsimd.ap_gather(gout[:], winD[:], idxD[:],
                                            channels=128, num_elems=DW_E, d=1,
                                            num_idxs=8192)
                        nc.vector.scalar_tensor_tensor(accD[:], gout[:], float(w),
                                                       accD[:], op0=Alu.mult, op1=Alu.add)
                    # normalize: dog = refined accD / TOTD  (in place)
                    nc.vector.tensor_scalar(gout[:], accD[:], float(INV_TOTD), None,
                                            op0=Alu.mult)
                    nc.vector.scalar_tensor_tensor(accD[:], gout[:], float(TOTD),
                                                   accD[:], op0=Alu.mult, op1=Alu.subtract)
                    nc.vector.scalar_tensor_tensor(accD[:], accD[:], -float(INV_TOTD),
                                                   gout[:], op0=Alu.mult, op1=Alu.add)
                    # write dog to fpl comp 2 via lane-0 view (one DMA per col lane)
                    srcv = accD[:].rearrange("(g q) (s c) -> q g s c", q=16, c=16)[0]
                    for c2 in range(16):
                        dst = AP(tensor=fpl_t,
                                 offset=(128 * st + c2) * FPL_R * 3
                                 + (30 + rb) * 3 + 2,
                                 ap=[[16 * FPL_R * 3, 8], [3, 512]])
                        nc.sync.dma_start(dst, srcv[:, :, c2])

        if DBG:
            with tc.tile_pool(name="dbg2", bufs=1) as pool:
                dt2 = pool.tile([128, FPL_R * 3], f32, name="dbg2_t", tag="dbg2_t")
                for k2 in range(3):
                    nc.sync.dma_start(
                        dt2[:], fpl[128 * k2:128 * (k2 + 1), :, :]
                        .rearrange("p r d -> p (r d)"))
                    nc.sync.dma_start(
                        dfpl_o[128 * k2:128 * (k2 + 1), :, :]
                        .rearrange("p r d -> p (r d)"), dt2[:])

        # ---------------- stage 4: streamline + threshold ----------------
        # Two direction chains with private state, steps interleaved so the
        # GPSIMD queue (scatter+gather) stays busy while the other chain's
        # vector math runs.
        with tc.tile_pool(name="str", bufs=1) as pool:
            win3 = pool.tile([128, SW_E * 3], f32, name="win3", tag="win3")
            g3 = [pool.tile([128, 1024 * 3], f32, name=f"g3{d}", tag=f"g3{d}")
                  for d in range(2)]
            p0t = [pool.tile([128, 1024], f32, name=f"p0t{d}", tag=f"p0t{d}")
                   for d in range(2)]
            p1t = [pool.tile([128, 1024], f32, name=f"p1t{d}", tag=f"p1t{d}")
                   for d in range(2)]
            accs = [pool.tile([128, 1024], f32, name=f"acc{d}", tag=f"acc{d}")
                    for d in range(2)]
            trS = [pool.tile([128, 1024], f32, name=f"trS{d}", tag=f"trS{d}")
                   for d in range(2)]
            tcS = [pool.tile([128, 1024], f32, name=f"tcS{d}", tag=f"tcS{d}")
                   for d in range(2)]
            li16 = [pool.tile([128, 1024], i16, name=f"li16{d}", tag=f"li16{d}")
                    for d in range(2)]
            idx16 = [pool.tile([128, 64], i16, name=f"idx16{d}", tag=f"idx16{d}")
                     for d in range(2)]
            rg0f = pool.tile([128, 1024], f32, name="rg0f", tag="rg0f")
            cgrid = pool.tile([128, 1024], f32, name="cgr", tag="cgr")
            id64 = pool.tile([128, 64], f32, name="id64", tag="id64")
            idxid = pool.tile([128, 64], i16, name="idxid", tag="idxid")
            spatT = pool.tile([128, 1024], i16, name="spatT", tag="spatT")
            mrep = pool.tile([128, 1024], u8, name="mrep", tag="mrep")
            ccS = pool.tile([128, 20], f32, name="ccS", tag="ccS")

            nc.sync.dma_start(ccS[:], cc)
            nc.sync.dma_start(spatT[:], spat)
            iq = trS[0][:].bitcast(i32)
            # rg0f[p, i] = i//16
            nc.gpsimd.iota(iq.rearrange("p (s q) -> p s q", q=16),
                           pattern=[[1, 64], [0, 16]], base=0, channel_multiplier=0)
            nc.vector.tensor_copy(rg0f[:], iq)
            # id64[p, s] = (p%16+30)*124 + 30 + s
            nc.gpsimd.iota(iq[:, 0:64], pattern=[[1, 64]], base=0,
                           channel_multiplier=0)
            nc.vector.tensor_copy(id64[:], iq[:, 0:64])
            nc.vector.tensor_scalar(id64[:], id64[:], ccS[:, 16:17], None, op0=Alu.add)
            nc.vector.tensor_copy(idxid[:], id64[:])

            g3v = [g3[d][:].rearrange("p (i d) -> p i d", d=3) for d in range(2)]
            n_steps = int(os.environ.get("BASSK_STEPS", MAX_S))
            SDIR = (-1.0, 1.0)

            for h in range(2):
                soff_ap = ccS[:, 14 + h:15 + h]
                # cgrid = (i%16) + scb(h): iota into scratch, add per-partition base
                nc.gpsimd.iota(trS[0][:].bitcast(i32).rearrange(
                    "p (s q) -> p s q", q=16),
                    pattern=[[0, 64], [1, 16]], base=0, channel_multiplier=0)
                nc.vector.tensor_copy(tcS[0][:], trS[0][:].bitcast(i32))
                nc.vector.tensor_scalar(cgrid[:], tcS[0][:], ccS[:, 12 + h:13 + h],
                                        None, op0=Alu.add)
                for sl in range(16):
                    rb = 64 * sl
                    # stage fused window (shared, read-only for both chains)
                    wv = win3[:].rearrange("(g q) e -> q g e", q=16)
                    for q in range(16):
                        sap = AP(tensor=fpl_t,
                                 offset=(128 * h * FPL_R + rb) * 3,
                                 ap=[[16 * FPL_R * 3, 8], [FPL_R * 3, 76],
                                     [1, 124 * 3]])
                        nc.sync.dma_start(wv[q], sap)
                    for d in range(2):
                        nc.vector.memset(accs[d][:], 0.0)
                        nc.vector.tensor_scalar(p0t[d][:], rg0f[:], float(rb),
                                                None, op0=Alu.add)
                        nc.vector.tensor_copy(p1t[d][:], cgrid[:])
                        nc.gpsimd.ap_gather(g3[d][:], win3[:], idxid[:],
                                            channels=128, num_elems=SW_E, d=3,
                                            num_idxs=1024)
                    for stp in range(n_steps):
                        for d in range(2):
                            sdir = SDIR[d]
                            if stp > 0:
                                # apply the PREVIOUS step's dog tap before g3 is
                                # overwritten (deferred so the vector queue does
                                # not stall behind the other chain's gather)
                                nc.vector.scalar_tensor_tensor(
                                    accs[d][:], g3v[d][:, :, 2],
                                    float(WS_S[stp - 1]), accs[d][:],
                                    op0=Alu.mult, op1=Alu.add)
                            nc.vector.scalar_tensor_tensor(
                                p0t[d][:], g3v[d][:, :, 0], float(sdir), p0t[d][:],
                                op0=Alu.mult, op1=Alu.add)
                            nc.vector.scalar_tensor_tensor(
                                p1t[d][:], g3v[d][:, :, 1], float(sdir), p1t[d][:],
                                op0=Alu.mult, op1=Alu.add)
                            nc.vector.tensor_scalar(p0t[d][:], p0t[d][:], 0.0,
                                                    1023.0, op0=Alu.max, op1=Alu.min)
                            nc.vector.tensor_scalar(p1t[d][:], p1t[d][:], 0.0,
                                                    1023.0, op0=Alu.max, op1=Alu.min)
                            nc.vector.tensor_scalar(trS[d][:], p0t[d][:], CBIG,
                                                    CBIG, op0=Alu.add,
                                                    op1=Alu.subtract)
                            nc.vector.tensor_scalar(tcS[d][:], p1t[d][:], CBIG,
                                                    CBIG, op0=Alu.add,
                                                    op1=Alu.subtract)
                            nc.vector.scalar_tensor_tensor(
                                trS[d][:], tcS[d][:], 124.0, trS[d][:],
                                op0=Alu.mult, op1=Alu.add)
                            nc.vector.tensor_scalar(trS[d][:], trS[d][:], soff_ap,
                                                    float(rb - 30),
                                                    op0=Alu.subtract,
                                                    op1=Alu.subtract)
                            nc.vector.tensor_copy(li16[d][:], trS[d][:])
                            nc.gpsimd.local_scatter(idx16[d][:], li16[d][:],
                                                    spatT[:], channels=128,
                                                    num_elems=64, num_idxs=1024)
                            nc.gpsimd.ap_gather(g3[d][:], win3[:], idx16[d][:],
                                                channels=128, num_elems=SW_E,
                                                d=3, num_idxs=1024)
                    for d in range(2):
                        nc.vector.scalar_tensor_tensor(
                            accs[d][:], g3v[d][:, :, 2], float(WS_S[n_steps - 1]),
                            accs[d][:], op0=Alu.mult, op1=Alu.add)
                    # finale: identity dog, threshold, write mask rows
                    nc.gpsimd.ap_gather(g3[0][:], win3[:], idxid[:],
                                        channels=128, num_elems=SW_E, d=3,
                                        num_idxs=1024)
                    nc.vector.scalar_tensor_tensor(trS[0][:], g3v[0][:, :, 2],
                                                   float(W0_S), accs[0][:],
                                                   op0=Alu.mult, op1=Alu.add)
                    nc.vector.tensor_add(trS[0][:], trS[0][:], accs[1][:])
                    nc.vector.tensor_scalar(mrep[:], trS[0][:], float(CUT_ACC),
                                            None, op0=Alu.is_ge)
                    src = mrep[:].rearrange("(g q) i -> q g i", q=16)[0]
                    nc.sync.dma_start(mask_o[128 * h + 8 * sl:128 * h + 8 * sl + 8, :],
                                      src)

    nc.compile()
    return nc


def _get_nc():
    if "nc" not in _CACHE:
        _CACHE["nc"] = _build()
    return _CACHE["nc"]


# ================================================================ host side
def _host_inputs(images):
    """Build per-core in_maps."""
    images = np.asarray(images, dtype=np.float32)
    scat = np.full((128, 1024), -1, np.int16)
    for p in range(128):
        for s in range(64):
            scat[p, s * 16 + p % 16] = s
    in_maps = []
    for core in range(N_CORES):
        b = core // CPI
        c0 = (core % CPI) * 256
        s0 = c0 - 40
        im = np.zeros((SLAB, IMROWS), np.float32)
        lo = max(0, s0)
        hi = min(Y, s0 + SLAB)
        im[lo - s0:hi - s0, 9:1033] = images[b, 0, :, lo:hi].T
        ccm = np.zeros((128, 20), np.float32)
        p = np.arange(128)
        for st in range(3):
            clo_g = s0 + np.minimum(128 * st + 16 * (p // 16) + 1, 350)
            ccm[:, st] = clo_g
            ccm[:, 3 + st] = clo_g + 33
            ccm[:, 6 + st] = clo_g * 530
            ccm[:, 9 + st] = c0 - 30 + 128 * st + p
        for h in range(2):
            cb = c0 + 128 * h + 16 * (p // 16)
            ccm[:, 12 + h] = cb
            ccm[:, 14 + h] = (cb - 30) * 124
        ccm[:, 16] = (p % 16 + 30) * 124 + 30
        in_maps.append({"img": im, "cconst": ccm, "scatpat": scat})
    return in_maps


def _decode_mask(res):
    out = np.empty((B, 1, X, Y), np.int32)
    for core in range(N_CORES):
        b = core // CPI
        c0 = (core % CPI) * 256
        m = res[core]["mask"].reshape(2, 16, 8, 64, 16)  # h, sl, g, s, q
        blk = np.transpose(m, (1, 3, 0, 2, 4)).reshape(1024, 256)  # row, col
        out[b, 0, :, c0:c0 + 256] = (blk != 0).astype(np.int32)
    return out


def kernel(images: np.ndarray) -> np.ndarray:
    from concourse.bass_utils import run_bass_kernel_spmd

    nc = _get_nc()
    in_maps = _host_inputs(images)
    t_dev = time.time()
    res = run_bass_kernel_spmd(nc, in_maps, core_ids=list(range(N_CORES)))
    _CACHE["device_wall_ns"] = int((time.time() - t_dev) * 1e9)
    if res.exec_time_ns:
        _CACHE["exec_time_ns"] = res.exec_time_ns
    _CACHE["last_res"] = res.results
    return _decode_mask(res.results)


# revision 6
# speedup vs baseline: 6.9646x; 6.9646x over previous
"""FDoG fully on-device for Trainium2 (8 cores, column-parallel).

Everything in [partition=column, free=row] layout. Per core: a 384-col slab
(halo included) arrives as input; sobel/tang/smag (Newton-refined sqrt and
divisions), the 6-step ETF relaxation, the 19-tap DoG (GPSIMD ap_gather over
per-16-col-group image windows), the 2x30-step streamline integration (fused
(etfx,etfy,dog) d=3 gathers, replicated state), and the final threshold all
run on device.  Output: u8 mask, decoded and assembled on host.
"""

import math
import os
import time

import numpy as np

# ---------------------------------------------------------------- constants
MU = 10
ITERATIONS = 3
SIGMA_C = 3.0
SIGMA_S = SIGMA_C * 1.6
SIGMA_M = 10.0
RHO = 0.99
DELTA = 1.0
MAX_T = 9
MAX_S = 30

B, X, Y = 2, 1024, 1024
N_CORES = 8
CPI = 4

F = 1064                # ETF tile free dim (rows + zero tail)
RV = 1034               # ETF compute rows
NT = 3                  # slab partition tiles
SLAB = 384              # slab cols;  s0 = c0 - 40
IMROWS = 1042           # img input rows: global rows -9..1032

FPL_C, FPL_R = 384, 1120          # fused plane: col c0-30+P, row R-30
DW_E = 34 * 530                   # DoG window elems (34 cols x 530 rows)
SW_E = 76 * 124                   # streamline window elems (76 x 124)
CBIG = 2.0 ** 23


def _gauss(v, sigma):
    return math.exp(-v ** 2 / (2.0 * sigma ** 2)) / (math.sqrt(2.0 * math.pi) * sigma)


W0_S = np.float32(_gauss(0, SIGMA_M))
WS_S = [np.float32(_gauss(s, SIGMA_M)) for s in range(1, MAX_S + 1)]
WT_D = [np.float32(_gauss(t, SIGMA_C) - RHO * _gauss(t, SIGMA_S))
        for t in range(-MAX_T, MAX_T + 1)]
TOTD = np.float32(sum(_gauss(t, SIGMA_C) - RHO * _gauss(t, SIGMA_S)
                      for t in range(-MAX_T, MAX_T + 1)))
INV_TOTD = np.float32(1.0 / np.float64(TOTD))
CUT_ACC = np.uint32(0xbe9e1cea).view(np.float32)  # acc>=CUT <=> keep (bisected)

_CACHE = {}
DBG = bool(os.environ.get("BASSK_DBG"))


# ================================================================ bass build
def _build():
    import concourse.bacc as bacc
    import concourse.mybir as mybir
    import concourse.tile as tile
    from concourse.bass_types import AP

    f32 = mybir.dt.float32
    i16 = mybir.dt.int16
    i32 = mybir.dt.int32
    u8 = mybir.dt.uint8
    Alu = mybir.AluOpType
    Act = mybir.ActivationFunctionType
    Ax = mybir.AxisListType

    nc = bacc.Bacc("TRN2", target_bir_lowering=False, debug=False,
                   enable_asserts=False, num_devices=N_CORES)

    img = nc.dram_tensor("img", [SLAB, IMROWS], f32, kind="ExternalInput").ap()
    cc = nc.dram_tensor("cconst", [128, 20], f32, kind="ExternalInput").ap()
    spat = nc.dram_tensor("scatpat", [128, 1024], i16, kind="ExternalInput").ap()
    mask_o = nc.dram_tensor("mask", [256, 1024], u8, kind="ExternalOutput").ap()
    if DBG:
        dtng_o = nc.dram_tensor("dtng", [3 * SLAB, F], f32, kind="ExternalOutput").ap()
        dfpl_o = nc.dram_tensor("dfpl", [FPL_C, FPL_R, 3], f32, kind="ExternalOutput").ap()

    tng = nc.dram_tensor("tng", [3 * SLAB, F], f32, kind="Internal").ap()
    fpl_t = nc.dram_tensor("fpl", [FPL_C, FPL_R, 3], f32, kind="Internal")
    fpl = fpl_t.ap()
    grin = nc.dram_tensor("grin", [1, 1], f32, kind="Internal").ap()
    grout = nc.dram_tensor("grout", [1, 1], f32, kind="Internal").ap()
    gtmp = nc.dram_tensor("gtmp", [3, 128], f32, kind="Internal").ap()
    gbd = nc.dram_tensor("gbd", [2, 128], f32, kind="Internal").ap()

    with tile.TileContext(nc) as tc:
        # ---------------- stage 1: sobel, mag, tang0, smag ----------------
        with tc.tile_pool(name="s1", bufs=1) as pool:
            imt = [pool.tile([128, F], f32, name=f"imt{t}", tag=f"imt{t}") for t in range(NT)]
            imL = [pool.tile([128, F], f32, name=f"imL{t}", tag=f"imL{t}") for t in range(NT)]
            imR = [pool.tile([128, F], f32, name=f"imR{t}", tag=f"imR{t}") for t in range(NT)]
            gx = [pool.tile([128, F], f32, name=f"gx{t}", tag=f"gx{t}") for t in range(NT)]
            gy = [pool.tile([128, F], f32, name=f"gy{t}", tag=f"gy{t}") for t in range(NT)]
            ta = [pool.tile([128, F], f32, name=f"ta{t}", tag=f"ta{t}") for t in range(NT)]
            tb = [pool.tile([128, F], f32, name=f"tb{t}", tag=f"tb{t}") for t in range(NT)]
            tcl = [pool.tile([128, F], f32, name=f"tc{t}", tag=f"tc{t}") for t in range(NT)]
            sc = [pool.tile([128, F], f32, name=f"sc{k}", tag=f"sc{k}") for k in range(6)]
            mx = [pool.tile([128, 1], f32, name=f"mx{t}", tag=f"mx{t}") for t in range(NT)]
            gall = pool.tile([1, 384], f32, name="gall", tag="gall")
            g1 = pool.tile([1, 1], f32, name="g1", tag="g1")
            gsc = pool.tile([1, 4], f32, name="gsc", tag="gsc")
            gb = pool.tile([1, 256], f32, name="gb", tag="gb")
            bc = pool.tile([128, 4], f32, name="bc", tag="bc")

            for t in range(NT):
                nc.vector.memset(imt[t][:], 0.0)
                nc.vector.memset(imL[t][:], 0.0)
                nc.vector.memset(imR[t][:], 0.0)
                nc.vector.memset(gx[t][:], 0.0)
                nc.vector.memset(gy[t][:], 0.0)
                # rows -1..1024 at free 0..1025  (img free = row+9)
                nc.sync.dma_start(imt[t][:, 0:1026],
                                  img[128 * t:128 * (t + 1), 8:1034])
            # col-shifted copies: imL[p]=img col-1, imR[p]=img col+1
            for t in range(NT):
                nc.sync.dma_start(imL[t][1:128, 0:1026], imt[t][0:127, 0:1026])
                if t > 0:
                    nc.sync.dma_start(imL[t][0:1, 0:1026], imt[t - 1][127:128, 0:1026])
                nc.sync.dma_start(imR[t][0:127, 0:1026], imt[t][1:128, 0:1026])
                if t + 1 < NT:
                    nc.sync.dma_start(imR[t][127:128, 0:1026], imt[t + 1][0:1, 0:1026])

            v0 = slice(0, 1024)
            for t in range(NT):
                i0 = lambda x: x[t][:, 0:1024]
                i1 = lambda x: x[t][:, 1:1025]
                i2 = lambda x: x[t][:, 2:1026]
                A, Bt, C2 = ta[t], tb[t], tcl[t]
                # gx, numpy order
                nc.vector.scalar_tensor_tensor(A[:, v0], i0(imt), 2.0, i0(imL),
                                               op0=Alu.mult, op1=Alu.add)
                nc.vector.tensor_add(Bt[:, v0], A[:, v0], i0(imR))
                nc.vector.tensor_sub(A[:, v0], i2(imL), Bt[:, v0])
                nc.vector.scalar_tensor_tensor(Bt[:, v0], i2(imt), 2.0, A[:, v0],
                                               op0=Alu.mult, op1=Alu.add)
                nc.vector.tensor_add(gx[t][:, v0], Bt[:, v0], i2(imR))
                # gy
                nc.vector.scalar_tensor_tensor(A[:, v0], i1(imL), 2.0, i0(imL),
                                               op0=Alu.mult, op1=Alu.add)
                nc.vector.tensor_add(Bt[:, v0], A[:, v0], i2(imL))
                nc.vector.tensor_sub(A[:, v0], i0(imR), Bt[:, v0])
                nc.vector.scalar_tensor_tensor(Bt[:, v0], i1(imR), 2.0, A[:, v0],
                                               op0=Alu.mult, op1=Alu.add)
                nc.vector.tensor_add(gy[t][:, v0], Bt[:, v0], i2(imR))
                # mag = sqrt(gx^2+gy^2), Dekker-exact residual correction
                MUL = nc.vector.tensor_mul
                SUB = nc.vector.tensor_sub
                ADD = nc.vector.tensor_add
                TS = nc.vector.tensor_scalar
                STT = nc.vector.scalar_tensor_tensor
                s0v, s1v = sc[0][:, v0], sc[1][:, v0]
                Av, Bv, Cv = A[:, v0], Bt[:, v0], C2[:, v0]
                it_, il_, ir_ = imt[t][:, v0], imL[t][:, v0], imR[t][:, v0]
                MUL(it_, gx[t][:, v0], gx[t][:, v0])
                MUL(il_, gy[t][:, v0], gy[t][:, v0])
                ADD(Cv, it_, il_)                                   # C = m2
                TS(it_, Cv, 0.0, None, op0=Alu.is_equal)
                ADD(Bv, Cv, it_)                                    # B = m2z
                nc.scalar.activation(Av, Bv, Act.Sqrt)
                nc.vector.reciprocal(Av, Av)                        # r0
                MUL(it_, Bv, Av)
                STT(it_, it_, 0.5, Av, op0=Alu.mult, op1=Alu.mult)
                TS(it_, it_, -1.0, 1.5, op0=Alu.mult, op1=Alu.add)
                MUL(Av, Av, it_)                                    # A = r1
                MUL(it_, Cv, Av)                                    # imt = mag0
                # Dekker square of mag0
                TS(il_, it_, 4097.0, None, op0=Alu.mult)
                SUB(ir_, il_, it_)
                SUB(il_, il_, ir_)                                  # imL = hi
                SUB(ir_, it_, il_)                                  # imR = lo
                MUL(Bv, it_, it_)                                   # B = phi
                MUL(s0v, il_, il_)
                SUB(s0v, s0v, Bv)
                MUL(s1v, il_, ir_)
                TS(s1v, s1v, 2.0, None, op0=Alu.mult)
                ADD(s0v, s0v, s1v)
                MUL(s1v, ir_, ir_)
                ADD(s0v, s0v, s1v)                                  # s0 = err
                SUB(s1v, Cv, Bv)
                SUB(s1v, s1v, s0v)                                  # s1 = e exact
                STT(s1v, s1v, 0.5, Av, op0=Alu.mult, op1=Alu.mult)
                ADD(Cv, it_, s1v)                                   # C = mag
                nc.vector.memset(C2[:, 1024:F], 0.0)
                # col-max over rows
                nc.vector.tensor_reduce(mx[t][:, 0:1], C2[:, 0:1024],
                                        axis=Ax.X, op=Alu.max)
                nc.sync.dma_start(gtmp[t:t + 1, :], mx[t][:, 0:1])

            nc.sync.dma_start(gall[:], gtmp.rearrange("a b -> (a b)").unsqueeze(0))
            nc.vector.tensor_reduce(g1[:, 0:1], gall[:, 40:296], axis=Ax.X, op=Alu.max)
            nc.sync.dma_start(grin, g1[:, 0:1])
            if os.environ.get("BASSK_NOCC"):
                nc.sync.dma_start(grout, grin)
            else:
                nc.gpsimd.collective_compute(
                    "AllReduce", Alu.max,
                    replica_groups=[[0, 1, 2, 3, 4, 5, 6, 7]],
                    ins=[grin], outs=[grout])
            nc.sync.dma_start(gsc[:, 0:1], grout)
            # refined reciprocal of gmax on [1,1]
            nc.vector.reciprocal(gsc[:, 1:2], gsc[:, 0:1])
            nc.vector.tensor_mul(gsc[:, 2:3], gsc[:, 0:1], gsc[:, 1:2])
            nc.vector.tensor_scalar(gsc[:, 2:3], gsc[:, 2:3], -1.0, 2.0,
                                    op0=Alu.mult, op1=Alu.add)
            nc.vector.tensor_mul(gsc[:, 1:2], gsc[:, 1:2], gsc[:, 2:3])
            # broadcast gmax, rinv to 128 partitions via log-doubling + DRAM
            nc.vector.tensor_copy(gb[:, 0:1], gsc[:, 0:1])
            nc.vector.tensor_copy(gb[:, 128:129], gsc[:, 1:2])
            k = 1
            while k < 128:
                nc.vector.tensor_copy(gb[:, k:2 * k], gb[:, 0:k])
                nc.vector.tensor_copy(gb[:, 128 + k:128 + 2 * k], gb[:, 128:128 + k])
                k *= 2
            nc.sync.dma_start(gbd.rearrange("j p -> (j p)").unsqueeze(0), gb[:])
            nc.sync.dma_start(bc[:, 0:2], gbd.rearrange("j p -> p j"))
            gmax_ap = bc[:, 0:1]
            rinv_ap = bc[:, 1:2]
            # Veltkamp split of gmax -> bc[:,2]=ghi, bc[:,3]=glo
            nc.vector.tensor_scalar(bc[:, 2:3], bc[:, 0:1], 4097.0, None, op0=Alu.mult)
            nc.vector.tensor_sub(bc[:, 3:4], bc[:, 2:3], bc[:, 0:1])
            nc.vector.tensor_sub(bc[:, 2:3], bc[:, 2:3], bc[:, 3:4])
            nc.vector.tensor_sub(bc[:, 3:4], bc[:, 0:1], bc[:, 2:3])
            ghi_ap = bc[:, 2:3]
            glo_ap = bc[:, 3:4]

            MUL = nc.vector.tensor_mul
            SUB = nc.vector.tensor_sub
            ADD = nc.vector.tensor_add
            TS = nc.vector.tensor_scalar

            def dekker_div(q0, x, th, tl, tden, r, phi, qh, ql, er, e):
                """q0 <- correctly-rounded-ish x / tden (q0 = fl(x*r) on entry).
                th/tl: split of tden; phi,qh,ql,er,e: scratch views."""
                MUL(phi, q0, tden)
                TS(qh, q0, 4097.0, None, op0=Alu.mult)
                SUB(ql, qh, q0)
                SUB(qh, qh, ql)
                SUB(ql, q0, qh)
                MUL(er, qh, th)
                SUB(er, er, phi)
                MUL(e, qh, tl)
                ADD(er, er, e)
                MUL(e, ql, th)
                ADD(er, er, e)
                MUL(e, ql, tl)
                ADD(er, er, e)
                SUB(e, x, phi)
                SUB(e, e, er)
                MUL(e, e, r)
                ADD(q0, q0, e)

            for t in range(NT):
                A, Bt, C2 = ta[t], tb[t], tcl[t]
                Av, Bv, Cv = A[:, v0], Bt[:, v0], C2[:, v0]
                it_, il_, ir_ = imt[t][:, v0], imL[t][:, v0], imR[t][:, v0]
                s0v, s1v, s2v = sc[0][:, v0], sc[1][:, v0], sc[2][:, v0]
                s3v, s4v, s5v = sc[3][:, v0], sc[4][:, v0], sc[5][:, v0]
                # tmag = mag + (mag==0) -> B; CR reciprocal -> A
                TS(Av, Cv, 0.0, None, op0=Alu.is_equal)
                ADD(Bv, Cv, Av)
                nc.vector.reciprocal(Av, Bv)
                # split tmag -> imt=th, imL=tl
                TS(it_, Bv, 4097.0, None, op0=Alu.mult)
                SUB(il_, it_, Bv)
                SUB(it_, it_, il_)
                SUB(il_, Bv, it_)
                # t0y = gx/tmag -> imR
                MUL(ir_, gx[t][:, v0], Av)
                dekker_div(ir_, gx[t][:, v0], it_, il_, Bv, Av,
                           s0v, s1v, s2v, s3v, s4v)
                nc.vector.memset(imR[t][:, 1024:F], 0.0)
                nc.sync.dma_start(tng[SLAB + 128 * t:SLAB + 128 * (t + 1), :], imR[t][:])
                # t0x = (-gy)/tmag -> gy
                TS(s5v, gy[t][:, v0], -1.0, None, op0=Alu.mult)
                MUL(gy[t][:, v0], s5v, Av)
                dekker_div(gy[t][:, v0], s5v, it_, il_, Bv, Av,
                           s0v, s1v, s2v, s3v, s4v)
                nc.vector.memset(gy[t][:, 1024:F], 0.0)
                nc.sync.dma_start(tng[128 * t:128 * (t + 1), :], gy[t][:])
                # smag = mag/gmax -> A (phi->B via scalar mult)
                TS(Av, Cv, rinv_ap, None, op0=Alu.mult)          # q0
                TS(Bv, Av, gmax_ap, None, op0=Alu.mult)          # phi
                TS(s1v, Av, 4097.0, None, op0=Alu.mult)
                SUB(s2v, s1v, Av)
                SUB(s1v, s1v, s2v)                                # qh
                SUB(s2v, Av, s1v)                                 # ql
                TS(s3v, s1v, ghi_ap, None, op0=Alu.mult)
                SUB(s3v, s3v, Bv)
                TS(s4v, s1v, glo_ap, None, op0=Alu.mult)
                ADD(s3v, s3v, s4v)
                TS(s4v, s2v, ghi_ap, None, op0=Alu.mult)
                ADD(s3v, s3v, s4v)
                TS(s4v, s2v, glo_ap, None, op0=Alu.mult)
                ADD(s3v, s3v, s4v)                                # err
                SUB(s4v, Cv, Bv)
                SUB(s4v, s4v, s3v)                                # e exact
                TS(s4v, s4v, rinv_ap, None, op0=Alu.mult)
                ADD(Cv, Av, s4v)                                  # smag -> C
                nc.vector.memset(C2[:, 1024:F], 0.0)
                nc.sync.dma_start(tng[2 * SLAB + 128 * t:2 * SLAB + 128 * (t + 1), :],
                                  C2[:])

        if DBG:
            with tc.tile_pool(name="dbg1", bufs=1) as pool:
                dt_ = pool.tile([128, F], f32, name="dbg_t", tag="dbg_t")
                for k2 in range(9):
                    nc.sync.dma_start(dt_[:], tng[128 * k2:128 * (k2 + 1), :])
                    nc.sync.dma_start(dtng_o[128 * k2:128 * (k2 + 1), :], dt_[:])

        # ---------------- stage 2: ETF relaxation (baseline code) ---------
        with tc.tile_pool(name="etf", bufs=1) as pool:
            tgx = [pool.tile([128, F], f32, name=f"tgx{t}", tag=f"tgx{t}") for t in range(NT)]
            tgy = [pool.tile([128, F], f32, name=f"tgy{t}", tag=f"tgy{t}") for t in range(NT)]
            smg = [pool.tile([128, F], f32, name=f"smg{t}", tag=f"smg{t}") for t in range(NT)]
            smgs = [pool.tile([128, F], f32, name=f"smgs{t}", tag=f"smgs{t}") for t in range(NT)]
            nx = [pool.tile([128, F], f32, name=f"nx{t}", tag=f"nx{t}") for t in range(NT)]
            ny = [pool.tile([128, F], f32, name=f"ny{t}", tag=f"ny{t}") for t in range(NT)]
            sf = [pool.tile([128, F], f32, name=f"sf{t}", tag=f"sf{t}") for t in range(NT)]
            m2 = [pool.tile([128, F], f32, name=f"m2{t}", tag=f"m2{t}") for t in range(NT)]
            dts = [pool.tile([128, 4], f32, name=f"dt{t}", tag=f"dt{t}") for t in range(NT)]
            pp = [pool.tile([128, 8], f32, name=f"pp{t}", tag=f"pp{t}") for t in range(NT)]
            hx = [pool.tile([128, F], f32, name=f"hx{t}", tag=f"hx{t}") for t in range(NT)]
            hy = [pool.tile([128, F], f32, name=f"hy{t}", tag=f"hy{t}") for t in range(NT)]

            for t in range(NT):
                nc.vector.memset(nx[t][:], 0.0)
                nc.vector.memset(ny[t][:], 0.0)
                nc.vector.memset(smgs[t][:], 0.0)
                nc.vector.memset(hx[t][:], 0.0)
                nc.vector.memset(hy[t][:], 0.0)
                nc.sync.dma_start(tgx[t][:], tng[128 * t:128 * (t + 1), :])
                nc.sync.dma_start(tgy[t][:], tng[SLAB + 128 * t:SLAB + 128 * (t + 1), :])
                nc.sync.dma_start(smg[t][:], tng[2 * SLAB + 128 * t:2 * SLAB + 128 * (t + 1), :])

            def hshift(dst, src):
                for t in range(NT):
                    nc.sync.dma_start(dst[t][0:118, :], src[t][10:128, :])
                    if t + 1 < NT:
                        nc.sync.dma_start(dst[t][118:128, :], src[t + 1][0:10, :])

            hshift(smgs, smg)

            for _ in range(ITERATIONS):
                for ori in ("V", "H"):
                    if ori == "H":
                        hshift(hx, tgx)
                        hshift(hy, tgy)
                    for t in range(NT):
                        if ori == "V":
                            tYx = tgx[t][:, 10:10 + RV]
                            tYy = tgy[t][:, 10:10 + RV]
                            sY = smg[t][:, 10:10 + RV]
                        else:
                            tYx = hx[t][:, 0:RV]
                            tYy = hy[t][:, 0:RV]
                            sY = smgs[t][:, 0:RV]
                        v = slice(0, RV)
                        nc.vector.tensor_mul(m2[t][:, v], tgx[t][:, v], tYx)
                        nc.vector.tensor_reduce(
                            pp[t][:, 0:8],
                            m2[t][:, 0:1024].rearrange("p (a b) -> p a b", b=128),
                            axis=Ax.X, op=Alu.add)
                        nc.vector.tensor_reduce(
                            dts[t][:, 0:1], pp[t][:, 0:8], axis=Ax.X, op=Alu.add)
                        nc.vector.tensor_mul(m2[t][:, v], tgy[t][:, v], tYy)
                        nc.vector.tensor_reduce(
                            pp[t][:, 0:8],
                            m2[t][:, 0:1024].rearrange("p (a b) -> p a b", b=128),
                            axis=Ax.X, op=Alu.add)
                        nc.vector.tensor_reduce(
                            dts[t][:, 1:2], pp[t][:, 0:8], axis=Ax.X, op=Alu.add)
                        nc.vector.tensor_scalar_mul(dts[t][:, 0:2], dts[t][:, 0:2], 0.5)
                        nc.vector.tensor_sub(sf[t][:, v], sY, smg[t][:, v])
                        nc.vector.tensor_scalar_add(sf[t][:, v], sf[t][:, v], 1.0)
                        nc.vector.tensor_mul(nx[t][:, v], tYx, sf[t][:, v])
                        nc.vector.tensor_scalar_mul(nx[t][:, v], nx[t][:, v],
                                                    dts[t][:, 0:1])
                        nc.vector.tensor_mul(ny[t][:, v], tYy, sf[t][:, v])
                        nc.vector.tensor_scalar_mul(ny[t][:, v], ny[t][:, v],
                                                    dts[t][:, 1:2])
                        nc.vector.tensor_mul(m2[t][:, v], nx[t][:, v], nx[t][:, v])
                        nc.vector.tensor_mul(sf[t][:, v], ny[t][:, v], ny[t][:, v])
                        nc.vector.tensor_add(m2[t][:, v], m2[t][:, v], sf[t][:, v])
                    for t in range(NT):
                        v = slice(0, RV)
                        nc.scalar.activation(sf[t][:, v], m2[t][:, v], Act.Sqrt)
                        nc.vector.tensor_scalar(hx[t][:, v], sf[t][:, v], 0.0,
                                                None, op0=Alu.is_equal)
                        nc.vector.tensor_add(sf[t][:, v], sf[t][:, v], hx[t][:, v])
                        nc.vector.reciprocal(sf[t][:, v], sf[t][:, v])
                        nc.vector.tensor_mul(hx[t][:, v], m2[t][:, v], sf[t][:, v])
                        nc.vector.scalar_tensor_tensor(
                            hx[t][:, v], hx[t][:, v], 0.5, sf[t][:, v],
                            op0=Alu.mult, op1=Alu.mult)
                        nc.vector.tensor_scalar(hx[t][:, v], hx[t][:, v], -1.0,
                                                1.5, op0=Alu.mult, op1=Alu.add)
                        nc.vector.tensor_mul(m2[t][:, v], hx[t][:, v], sf[t][:, v])
                        nc.vector.tensor_mul(tgx[t][:, v], nx[t][:, v], m2[t][:, v])
                        nc.vector.tensor_mul(tgy[t][:, v], ny[t][:, v], m2[t][:, v])

            # write etf into fused plane: fpl[P, r+30, 0/1], P = slab_col - 10
            for t, (sp0, P0, npart) in enumerate(
                    [(10, 0, 118), (0, 118, 128), (0, 246, 128)]):
                for k2, tt in ((0, tgx), (1, tgy)):
                    dst = AP(tensor=fpl_t, offset=P0 * FPL_R * 3 + 30 * 3 + k2,
                             ap=[[FPL_R * 3, npart], [3, 1024]])
                    nc.sync.dma_start(dst, tt[t][sp0:sp0 + npart, 0:1024])

        # ---------------- stage 3: DoG -----------------------------------
        with tc.tile_pool(name="dog", bufs=1) as pool:
            winD = pool.tile([128, DW_E], f32, name="winD", tag="winD")
            accD = pool.tile([128, 8192], f32, name="accD", tag="accD")
            gout = pool.tile([128, 8192], f32, name="goutD", tag="goutD")
            exs = pool.tile([128, 1024], f32, name="exs", tag="exs")
            eys = pool.tile([128, 1024], f32, name="eys", tag="eys")
            rgn0 = pool.tile([128, 1024], i32, name="rgn0", tag="rgn0")
            rgn0f = pool.tile([128, 1024], f32, name="rgn0f", tag="rgn0f")
            cgn = pool.tile([128, 512], f32, name="cgn", tag="cgn")
            tr = pool.tile([128, 512], f32, name="trD", tag="trD")
            tcc = pool.tile([128, 512], f32, name="tcD", tag="tcD")
            lif = pool.tile([128, 512], f32, name="lifD", tag="lifD")
            idxD = pool.tile([128, 512], i16, name="idxD", tag="idxD")
            ccD = pool.tile([128, 20], f32, name="ccD", tag="ccD")

            nc.sync.dma_start(ccD[:], cc)
            nc.gpsimd.iota(rgn0[:], pattern=[[1, 1024]], base=0, channel_multiplier=0)
            nc.vector.tensor_copy(rgn0f[:], rgn0[:])

            for st in range(3):
                clo_ap = ccD[:, st:st + 1]
                chi_ap = ccD[:, 3 + st:4 + st]
                off_ap = ccD[:, 6 + st:7 + st]
                cg_ap = ccD[:, 9 + st:10 + st]
                # etf strips for per-pixel tangent
                nc.sync.dma_start(
                    exs[:], fpl[128 * st:128 * (st + 1), 30:1054, 0:1].squeeze(2))
                nc.sync.dma_start(
                    eys[:], fpl[128 * st:128 * (st + 1), 30:1054, 1:2].squeeze(2))
                nc.vector.tensor_scalar(cgn[:], rgn0f[:, 0:512], 0.0, cg_ap,
                                        op0=Alu.mult, op1=Alu.add)
                for sl in range(2):
                    rb = 512 * sl
                    # stage window: 16 lane-DMAs; group g: img[cbe(g), +34) x [rb, rb+530)
                    wv = winD[:].rearrange("(g q) e -> q g e", q=16)
                    uniform = (128 * st + 16 * 7 + 1) <= 350
                    for q in range(16):
                        if uniform:
                            sap = AP(tensor=img.tensor,
                                     offset=(128 * st + 1) * IMROWS + rb,
                                     ap=[[16 * IMROWS, 8], [IMROWS, 34], [1, 530]])
                            nc.sync.dma_start(wv[q], sap)
                        else:
                            # per-group clamped bases (st=2 edge groups)
                            for g in range(8):
                                be = min(128 * st + 16 * g + 1, 350)
                                s1 = AP(tensor=img.tensor, offset=be * IMROWS + rb,
                                        ap=[[IMROWS, 34], [1, 530]])
                                nc.sync.dma_start(wv[q][g:g + 1, :], s1.unsqueeze(0))
                    ex_v = exs[:, rb:rb + 512]
                    ey_v = eys[:, rb:rb + 512]
                    rg_v = rgn0f[:, rb:rb + 512]
                    nc.vector.memset(accD[:], 0.0)
                    for ti, t in enumerate(range(-MAX_T, MAX_T + 1)):
                        w = WT_D[ti]
                        # p0 = r + (-ey)*t ; p1 = c + ex*t
                        nc.vector.scalar_tensor_tensor(tr[:], ey_v, -float(t), rg_v,
                                                       op0=Alu.mult, op1=Alu.add)
                        nc.vector.scalar_tensor_tensor(tcc[:], ex_v, float(t), cgn[:],
                                                       op0=Alu.mult, op1=Alu.add)
                        nc.vector.tensor_scalar(tr[:], tr[:], 0.0, 1023.0,
                                                op0=Alu.max, op1=Alu.min)
                        nc.vector.tensor_scalar(tcc[:], tcc[:], 0.0, 1023.0,
                                                op0=Alu.max, op1=Alu.min)
                        nc.vector.tensor_scalar(tr[:], tr[:], CBIG, CBIG,
                                                op0=Alu.add, op1=Alu.subtract)
                        nc.vector.tensor_scalar(tcc[:], tcc[:], CBIG, CBIG,
                                                op0=Alu.add, op1=Alu.subtract)
                        nc.vector.tensor_scalar(tr[:], tr[:], float(rb - 9),
                                                float(rb + 520), op0=Alu.max, op1=Alu.min)
                        nc.vector.tensor_scalar(tcc[:], tcc[:], clo_ap, chi_ap,
                                                op0=Alu.max, op1=Alu.min)
                        nc.vector.scalar_tensor_tensor(lif[:], tcc[:], 530.0, tr[:],
                                                       op0=Alu.mult, op1=Alu.add)
                        nc.vector.tensor_scalar(lif[:], lif[:], off_ap, float(rb - 9),
                                                op0=Alu.subtract, op1=Alu.subtract)
                        nc.vector.tensor_copy(idxD[:], lif[:])
                        nc.gpsimd.ap_gather(gout[:], winD[:], idxD[:],
                                            channels=128, num_elems=DW_E, d=1,
                                            num_idxs=8192)
                        nc.vector.scalar_tensor_tensor(accD[:], gout[:], float(w),
                                                       accD[:], op0=Alu.mult, op1=Alu.add)
                    # normalize: dog = refined accD / TOTD  (in place)
                    nc.vector.tensor_scalar(gout[:], accD[:], float(INV_TOTD), None,
                                            op0=Alu.mult)
                    nc.vector.scalar_tensor_tensor(accD[:], gout[:], float(TOTD),
                                                   accD[:], op0=Alu.mult, op1=Alu.subtract)
                    nc.vector.scalar_tensor_tensor(accD[:], accD[:], -float(INV_TOTD),
                                                   gout[:], op0=Alu.mult, op1=Alu.add)
                    # write dog to fpl comp 2 via lane-0 view (one DMA per col lane)
                    srcv = accD[:].rearrange("(g q) (s c) -> q g s c", q=16, c=16)[0]
                    for c2 in range(16):
                        dst = AP(tensor=fpl_t,
                                 offset=(128 * st + c2) * FPL_R * 3
                                 + (30 + rb) * 3 + 2,
                                 ap=[[16 * FPL_R * 3, 8], [3, 512]])
                        nc.sync.dma_start(dst, srcv[:, :, c2])

        if DBG:
            with tc.tile_pool(name="dbg2", bufs=1) as pool:
                dt2 = pool.tile([128, FPL_R * 3], f32, name="dbg2_t", tag="dbg2_t")
                for k2 in range(3):
                    nc.sync.dma_start(
                        dt2[:], fpl[128 * k2:128 * (k2 + 1), :, :]
                        .rearrange("p r d -> p (r d)"))
                    nc.sync.dma_start(
                        dfpl_o[128 * k2:128 * (k2 + 1), :, :]
                        .rearrange("p r d -> p (r d)"), dt2[:])

        # ---------------- stage 4: streamline + threshold ----------------
        # Two direction chains with private state, steps interleaved so the
        # GPSIMD queue (scatter+gather) stays busy while the other chain's
        # vector math runs.
        with tc.tile_pool(name="str", bufs=1) as pool:
            win3 = pool.tile([128, SW_E * 3], f32, name="win3", tag="win3")
            g3 = [pool.tile([128, 1024 * 3], f32, name=f"g3{d}", tag=f"g3{d}")
                  for d in range(2)]
            p0t = [pool.tile([128, 1024], f32, name=f"p0t{d}", tag=f"p0t{d}")
                   for d in range(2)]
            p1t = [pool.tile([128, 1024], f32, name=f"p1t{d}", tag=f"p1t{d}")
                   for d in range(2)]
            accs = [pool.tile([128, 1024], f32, name=f"acc{d}", tag=f"acc{d}")
                    for d in range(2)]
            trS = [pool.tile([128, 1024], f32, name=f"trS{d}", tag=f"trS{d}")
                   for d in range(2)]
            tcS = [pool.tile([128, 1024], f32, name=f"tcS{d}", tag=f"tcS{d}")
                   for d in range(2)]
            li16 = [pool.tile([128, 1024], i16, name=f"li16{d}", tag=f"li16{d}")
                    for d in range(2)]
            idx16 = [pool.tile([128, 64], i16, name=f"idx16{d}", tag=f"idx16{d}")
                     for d in range(2)]
            rg0f = pool.tile([128, 1024], f32, name="rg0f", tag="rg0f")
            cgrid = pool.tile([128, 1024], f32, name="cgr", tag="cgr")
            id64 = pool.tile([128, 64], f32, name="id64", tag="id64")
            idxid = pool.tile([128, 64], i16, name="idxid", tag="idxid")
            spatT = pool.tile([128, 1024], i16, name="spatT", tag="spatT")
            mrep = pool.tile([128, 1024], u8, name="mrep", tag="mrep")
            ccS = pool.tile([128, 20], f32, name="ccS", tag="ccS")

            nc.sync.dma_start(ccS[:], cc)
            nc.sync.dma_start(spatT[:], spat)
            iq = trS[0][:].bitcast(i32)
            # rg0f[p, i] = i//16
            nc.gpsimd.iota(iq.rearrange("p (s q) -> p s q", q=16),
                           pattern=[[1, 64], [0, 16]], base=0, channel_multiplier=0)
            nc.vector.tensor_copy(rg0f[:], iq)
            # id64[p, s] = (p%16+30)*124 + 30 + s
            nc.gpsimd.iota(iq[:, 0:64], pattern=[[1, 64]], base=0,
                           channel_multiplier=0)
            nc.vector.tensor_copy(id64[:], iq[:, 0:64])
            nc.vector.tensor_scalar(id64[:], id64[:], ccS[:, 16:17], None, op0=Alu.add)
            nc.vector.tensor_copy(idxid[:], id64[:])

            g3v = [g3[d][:].rearrange("p (i d) -> p i d", d=3) for d in range(2)]
            n_steps = int(os.environ.get("BASSK_STEPS", MAX_S))
            SDIR = (-1.0, 1.0)

            for h in range(2):
                soff_ap = ccS[:, 14 + h:15 + h]
                # cgrid = (i%16) + scb(h): iota into scratch, add per-partition base
                nc.gpsimd.iota(trS[0][:].bitcast(i32).rearrange(
                    "p (s q) -> p s q", q=16),
                    pattern=[[0, 64], [1, 16]], base=0, channel_multiplier=0)
                nc.vector.tensor_copy(tcS[0][:], trS[0][:].bitcast(i32))
                nc.vector.tensor_scalar(cgrid[:], tcS[0][:], ccS[:, 12 + h:13 + h],
                                        None, op0=Alu.add)
                for sl in range(16):
                    rb = 64 * sl
                    # stage fused window (shared, read-only for both chains)
                    wv = win3[:].rearrange("(g q) e -> q g e", q=16)
                    for q in range(16):
                        sap = AP(tensor=fpl_t,
                                 offset=(128 * h * FPL_R + rb) * 3,
                                 ap=[[16 * FPL_R * 3, 8], [FPL_R * 3, 76],
                                     [1, 124 * 3]])
                        nc.sync.dma_start(wv[q], sap)
                    for d in range(2):
                        nc.vector.memset(accs[d][:], 0.0)
                        nc.vector.tensor_scalar(p0t[d][:], rg0f[:], float(rb),
                                                None, op0=Alu.add)
                        nc.vector.tensor_copy(p1t[d][:], cgrid[:])
                        nc.gpsimd.ap_gather(g3[d][:], win3[:], idxid[:],
                                            channels=128, num_elems=SW_E, d=3,
                                            num_idxs=1024)
                    for stp in range(n_steps):
                        for d in range(2):
                            sdir = SDIR[d]
                            if stp > 0:
                                # apply the PREVIOUS step's dog tap before g3 is
                                # overwritten (deferred so the vector queue does
                                # not stall behind the other chain's gather)
                                nc.vector.scalar_tensor_tensor(
                                    accs[d][:], g3v[d][:, :, 2],
                                    float(WS_S[stp - 1]), accs[d][:],
                                    op0=Alu.mult, op1=Alu.add)
                            nc.vector.scalar_tensor_tensor(
                                p0t[d][:], g3v[d][:, :, 0], float(sdir), p0t[d][:],
                                op0=Alu.mult, op1=Alu.add)
                            nc.vector.scalar_tensor_tensor(
                                p1t[d][:], g3v[d][:, :, 1], float(sdir), p1t[d][:],
                                op0=Alu.mult, op1=Alu.add)
                            nc.vector.tensor_scalar(p0t[d][:], p0t[d][:], 0.0,
                                                    1023.0, op0=Alu.max, op1=Alu.min)
                            nc.vector.tensor_scalar(p1t[d][:], p1t[d][:], 0.0,
                                                    1023.0, op0=Alu.max, op1=Alu.min)
                            nc.vector.tensor_scalar(trS[d][:], p0t[d][:], CBIG,
                                                    CBIG, op0=Alu.add,
                                                    op1=Alu.subtract)
                            nc.vector.tensor_scalar(tcS[d][:], p1t[d][:], CBIG,
                                                    CBIG, op0=Alu.add,
                                                    op1=Alu.subtract)
                            nc.vector.scalar_tensor_tensor(
                                trS[d][:], tcS[d][:], 124.0, trS[d][:],
                                op0=Alu.mult, op1=Alu.add)
                            nc.vector.tensor_scalar(trS[d][:], trS[d][:], soff_ap,
                                                    float(rb - 30),
                                                    op0=Alu.subtract,
                                                    op1=Alu.subtract)
                            nc.vector.tensor_copy(li16[d][:], trS[d][:])
                            nc.gpsimd.local_scatter(idx16[d][:], li16[d][:],
                                                    spatT[:], channels=128,
                                                    num_elems=64, num_idxs=1024)
                            nc.gpsimd.ap_gather(g3[d][:], win3[:], idx16[d][:],
                                                channels=128, num_elems=SW_E,
                                                d=3, num_idxs=1024)
                    for d in range(2):
                        nc.vector.scalar_tensor_tensor(
                            accs[d][:], g3v[d][:, :, 2], float(WS_S[n_steps - 1]),
                            accs[d][:], op0=Alu.mult, op1=Alu.add)
                    # finale: identity dog, threshold, write mask rows
                    nc.gpsimd.ap_gather(g3[0][:], win3[:], idxid[:],
                                        channels=128, num_elems=SW_E, d=3,
                                        num_idxs=1024)
                    nc.vector.scalar_tensor_tensor(trS[0][:], g3v[0][:, :, 2],
                                                   float(W0_S), accs[0][:],
                                                   op0=Alu.mult, op1=Alu.add)
                    nc.vector.tensor_add(trS[0][:], trS[0][:], accs[1][:])
                    nc.vector.tensor_scalar(mrep[:], trS[0][:], float(CUT_ACC),
                                            None, op0=Alu.is_ge)
                    src = mrep[:].rearrange("(g q) i -> q g i", q=16)[0]
                    nc.sync.dma_start(mask_o[128 * h + 8 * sl:128 * h + 8 * sl + 8, :],
                                      src)

    nc.compile()
    return nc


def _get_nc():
    if "nc" not in _CACHE:
        _CACHE["nc"] = _build()
    return _CACHE["nc"]


# ================================================================ host side
def _host_inputs(images):
    """Build per-core in_maps."""
    images = np.asarray(images, dtype=np.float32)
    scat = np.full((128, 1024), -1, np.int16)
    for p in range(128):
        for s in range(64):
            scat[p, s * 16 + p % 16] = s
    in_maps = []
    for core in range(N_CORES):
        b = core // CPI
        c0 = (core % CPI) * 256
        s0 = c0 - 40
        im = np.zeros((SLAB, IMROWS), np.float32)
        lo = max(0, s0)
        hi = min(Y, s0 + SLAB)
        im[lo - s0:hi - s0, 9:1033] = images[b, 0, :, lo:hi].T
        ccm = np.zeros((128, 20), np.float32)
        p = np.arange(128)
        for st in range(3):
            clo_g = s0 + np.minimum(128 * st + 16 * (p // 16) + 1, 350)
            ccm[:, st] = clo_g
            ccm[:, 3 + st] = clo_g + 33
            ccm[:, 6 + st] = clo_g * 530
            ccm[:, 9 + st] = c0 - 30 + 128 * st + p
        for h in range(2):
            cb = c0 + 128 * h + 16 * (p // 16)
            ccm[:, 12 + h] = cb
            ccm[:, 14 + h] = (cb - 30) * 124
        ccm[:, 16] = (p % 16 + 30) * 124 + 30
        in_maps.append({"img": im, "cconst": ccm, "scatpat": scat})
    return in_maps


def _decode_mask(res):
    out = np.empty((B, 1, X, Y), np.int32)
    for core in range(N_CORES):
        b = core // CPI
        c0 = (core % CPI) * 256
        m = res[core]["mask"].reshape(2, 16, 8, 64, 16)  # h, sl, g, s, q
        blk = np.transpose(m, (1, 3, 0, 2, 4)).reshape(1024, 256)  # row, col
        out[b, 0, :, c0:c0 + 256] = (blk != 0).astype(np.int32)
    return out


def _get_runner():
    """Cached jitted runner (mirrors bass2jax.run_bass_via_pjrt but keeps the
    jax.jit callable across invocations, avoiding a full re-trace per call)."""
    if "runner" in _CACHE:
        return _CACHE["runner"]
    import jax
    import numpy as _np
    from jax.sharding import Mesh, PartitionSpec
    from jax.experimental.shard_map import shard_map
    import concourse.mybir as mybir
    from concourse import bass2jax

    nc = _get_nc()
    bass2jax.install_neuronx_cc_hook()
    partition_name = (nc.partition_id_tensor.name
                      if nc.partition_id_tensor else None)
    in_names, out_names, out_avals, zero_outs = [], [], [], []
    for alloc in nc.m.functions[0].allocations:
        if not isinstance(alloc, mybir.MemoryLocationSet):
            continue
        name = alloc.memorylocations[0].name
        if alloc.kind == "ExternalInput":
            if name != partition_name:
                in_names.append(name)
        elif alloc.kind == "ExternalOutput":
            shape = tuple(alloc.tensor_shape)
            dtype = mybir.dt.np(alloc.dtype)
            out_avals.append(jax.core.ShapedArray(shape, dtype))
            out_names.append(name)
            zero_outs.append(_np.zeros(shape, dtype))
    n_params = len(in_names)
    n_outs = len(out_avals)
    all_names = list(in_names) + list(out_names)
    if partition_name is not None:
        all_names.append(partition_name)
    donate = tuple(range(n_params, n_params + n_outs))

    def _body(*args):
        operands = list(args)
        if partition_name is not None:
            operands.append(bass2jax.partition_id_tensor())
        outs = bass2jax._bass_exec_p.bind(
            *operands,
            out_avals=tuple(out_avals),
            in_names=tuple(all_names),
            out_names=tuple(out_names),
            lowering_input_output_aliases=(),
            sim_require_finite=True,
            sim_require_nnan=True,
            nc=nc,
        )
        return tuple(outs)

    devices = jax.devices()[:N_CORES]
    mesh = Mesh(_np.asarray(devices), ("core",))
    in_specs = (PartitionSpec("core"),) * (n_params + n_outs)
    out_specs = (PartitionSpec("core"),) * n_outs
    sharded = jax.jit(
        shard_map(_body, mesh=mesh, in_specs=in_specs, out_specs=out_specs,
                  check_rep=False),
        donate_argnums=donate, keep_unused=True)

    def run(in_maps):
        concat_in = [
            _np.concatenate([_np.asarray(in_maps[c][nm]) for c in range(N_CORES)],
                            axis=0)
            for nm in in_names]
        concat_zeros = [
            _np.zeros((N_CORES * z.shape[0], *z.shape[1:]), z.dtype)
            for z in zero_outs]
        out_arrs = sharded(*concat_in, *concat_zeros)
        return [
            {nm: _np.asarray(out_arrs[i]).reshape(N_CORES, *out_avals[i].shape)[c]
             for i, nm in enumerate(out_names)}
            for c in range(N_CORES)]

    _CACHE["runner"] = run
    return run


def kernel(images: np.ndarray) -> np.ndarray:
    run = _get_runner()
    in_maps = _host_inputs(images)
    t_dev = time.time()
    results = run(in_maps)
    _CACHE["device_wall_ns"] = int((time.time() - t_dev) * 1e9)
    _CACHE["last_res"] = results
    return _decode_mask(results)
